# revision 40
# baseline (speedup 1.0000x reference)
"""GATv2 (2-layer, 8-head) Trainium2 kernel, 8-core node-sharded.

v2 design (host-expanded slabs, no device-side gather):

  T1 NEFF:   per-core node transform xl/xr = x@W{l,r}+b via bf16 matmuls.
  host:      assembles xl by node id, expands the per-edge slab
             slab[d, off_j+q, :] = xl[src of q-th in-edge of node at block
             slot (j, d)] (pure data movement), so the edge NEFF streams it
             densely at full HBM bandwidth instead of paying ~10ns/row of
             GPSIMD descriptor-generation ucode for a gathering DMA.
  E1 NEFF:   per-edge score (DVE add -> Act leaky-relu -> DVE mult ->
             DVE segmented reduce), segment softmax without max-subtraction
             (scores are O(10), fp32 exp is safe), exp-weighted aggregation
             via paired identity matmuls accumulating in PSUM, ELU, plus the
             fused layer-2 node transform (PE transpose + matmul) emitting
             xl2/xr2 as a [128, NOWN] feature-major tensor.
  host:      expands slab2 from xl2 rows, transposes xr2.
  E2 NEFF:   same edge pipeline at F=64 with quad identity matmuls and the
             log-softmax tail (ln via exponent/mantissa polynomial).

Edges are laid out stratum-major: edge slot (q, d) of block j holds the
q-th in-edge of the node at partition d, so the xr broadcast is a plain
broadcast AP, and segment max/sum are free-dim reduces. All cores share the
per-slot stratum counts G[j] so a single NEFF serves all 8 cores (SPMD).
"""

import os
from contextlib import ExitStack

import ml_dtypes
import numpy as np

N, E0, DIN, H, DH, DOUT = 50000, 1600000, 128, 8, 16, 7
F1 = H * DH            # 128
C2 = 8                 # layer-2 per-head cols in slab (7 real + 1 pad)
F2P = H * C2           # 64
NCORES = 8
P = 128
NBLK = 392             # 392*128 = 50176 >= N, 392 % 8 == 0
NB = NBLK // NCORES    # 49 blocks per core
NOWN = NB * P          # 6272 nodes per core (incl. pad slots)
NPAD = NBLK * P        # 50176
FM1 = F1 + H           # 136 (agg | denom)
FM2 = F2P + H          # 72
FOUT = H * DOUT        # 56
NEG = -1.0e9
EPS = 1e-16
TBATCH = 8             # blocks per batched DMA

_f32 = np.float32
_bf16 = ml_dtypes.bfloat16

# tuning switches (validated on hardware)
LRELU_MODE = "prelu"   # "prelu" (Act) | "abs" (Act Abs + DVE add) | "stt" (DVE)
MS_SPLIT = 48          # feature cols 0:MS_SPLIT of Ms on DVE, rest on Pool
SC_BF16 = False        # bf16-out reduce is broken on HW (measured) - keep f32
BUFS_SLAB = 3          # pipeline depth for slab-sized tiles
BUFS_SMALL = 3


# ---------------------------------------------------------------------------
# host-side graph preprocessing (pure index/layout manipulation)
# ---------------------------------------------------------------------------

def _prep_graph(edge_index):
    src = np.concatenate([edge_index[0], np.arange(N, dtype=np.int64)])
    dst = np.concatenate([edge_index[1], np.arange(N, dtype=np.int64)])
    deg = np.bincount(dst, minlength=N).astype(np.int64)

    # group nodes into blocks of 128 with near-equal degree
    order = np.argsort(deg, kind="stable")
    nodes_sorted = np.concatenate([order, np.full(NPAD - N, -1, np.int64)])
    blocks = nodes_sorted.reshape(NBLK, P)          # [392, 128]
    blkmax = np.where(blocks >= 0, deg[np.maximum(blocks, 0)], 0).max(axis=1)
    G = blkmax.reshape(NB, NCORES).max(axis=1).astype(int)   # [49] shared
    G = np.maximum(G, 1)
    off = np.concatenate([[0], np.cumsum(G)]).astype(int)
    sumG = int(off[-1])

    # per-node padded src lists
    oe = np.argsort(dst, kind="stable")
    ss = dst[oe]
    starts = np.searchsorted(ss, np.arange(N))
    pos = np.arange(len(ss)) - starts[ss]
    Amax = max(int(deg.max()), int(G.max()))
    pad = np.zeros((N, Amax), np.int32)
    pad[ss, pos] = src[oe]

    members = [None] * NCORES
    idx = [None] * NCORES
    mneg = [None] * NCORES
    for k in range(NCORES):
        mem = blocks[np.arange(NB) * NCORES + k]   # [49, 128]
        members[k] = mem
        ia = np.zeros((P, sumG), np.int32)
        mg = np.full((P, sumG), NEG, _f32)
        for j in range(NB):
            g = G[j]
            m = mem[j]
            msafe = np.maximum(m, 0)
            darr = np.where(m >= 0, deg[msafe], 0)
            ia[:, off[j]:off[j] + g] = pad[msafe, :g]
            mg[:, off[j]:off[j] + g] = np.where(
                np.arange(g)[None, :] < darr[:, None], 0.0, NEG)
        idx[k], mneg[k] = ia, mg

    return dict(members=members, G=G, off=off, sumG=sumG, idx=idx, mneg=mneg)


# ---------------------------------------------------------------------------
# NEFF builders
# ---------------------------------------------------------------------------

def _mk_bass():
    import concourse.bacc as bacc
    return bacc.Bacc("TRN2", target_bir_lowering=False)


def _build_t1():
    """xT [128, NOWN] bf16 @ Wcat [128, 256] -> xl rows + xr rows (bf16)."""
    import concourse.mybir as mybir
    import concourse.tile as tile

    nc = _mk_bass()
    dt = mybir.dt
    op = mybir.AluOpType
    fo = 2 * F1

    xT = nc.dram_tensor("xT", [P, NOWN], dt.bfloat16, kind="ExternalInput")
    W = nc.dram_tensor("Wcat", [P, fo], dt.bfloat16, kind="ExternalInput")
    B = nc.dram_tensor("Bcat", [P, fo], dt.float32, kind="ExternalInput")
    xl = nc.dram_tensor("xl", [NOWN, F1], dt.bfloat16, kind="ExternalOutput")
    xr = nc.dram_tensor("xr", [NOWN, F1], dt.bfloat16, kind="ExternalOutput")

    with tile.TileContext(nc) as tc, ExitStack() as ctx:
        const = ctx.enter_context(tc.tile_pool(name="const", bufs=1))
        work = ctx.enter_context(tc.tile_pool(name="work", bufs=2))
        psum = ctx.enter_context(tc.tile_pool(name="psum", bufs=3, space="PSUM"))

        w_s = const.tile([P, fo], dt.bfloat16)
        nc.sync.dma_start(w_s[:], W[:, :])
        b_s = const.tile([P, fo], dt.float32)
        nc.sync.dma_start(b_s[:], B[:, :])

        for jj in range(0, NB, TBATCH):
            nb = min(TBATCH, NB - jj)
            lhs = work.tile([P, nb, P], dt.bfloat16, tag="lhs")
            nc.sync.dma_start(lhs[:], xT[:, jj * P:(jj + nb) * P])
            ol = work.tile([P, nb, F1], dt.bfloat16, tag="ol")
            orr = work.tile([P, nb, F1], dt.bfloat16, tag="orr")
            for b in range(nb):
                ps = psum.tile([P, fo], dt.float32, tag="ps")
                nc.tensor.matmul(ps[:], lhs[:, b, :], w_s[:],
                                 start=True, stop=True)
                nc.vector.tensor_tensor(ol[:, b, :], ps[:, 0:F1],
                                        b_s[:, 0:F1], op.add)
                nc.vector.tensor_tensor(orr[:, b, :], ps[:, F1:fo],
                                        b_s[:, F1:fo], op.add)
            rows = slice(jj * P, (jj + nb) * P)
            nc.sync.dma_start(
                xl[rows, :].rearrange("(b d) f -> d b f", b=nb), ol[:])
            nc.sync.dma_start(
                xr[rows, :].rearrange("(b d) f -> d b f", b=nb), orr[:])
    nc.compile()
    return nc


def _edge_pipeline(nc, tc, ctx, layer, G, off, sumG):
    """Shared edge phase. Returns per-layer specifics handled by caller."""
    import concourse.mybir as mybir

    dt = mybir.dt
    op = mybir.AluOpType
    AF = mybir.ActivationFunctionType

    FU = F1 if layer == 1 else F2P
    C = DH if layer == 1 else C2
    FM = FU + H
    QSTEP = 2 if layer == 1 else 4   # matmul batch; QSTEP*FM*4B <= 2KB bank
    QCH = 48                         # edge strata per chunk (bounds tiles)

    slab_d = nc.dram_tensor("slab", [P, sumG, FU], dt.bfloat16,
                            kind="ExternalInput")
    xr_d = nc.dram_tensor("xr", [NOWN, FU], dt.bfloat16, kind="ExternalInput")
    mneg_d = nc.dram_tensor("mneg", [P, sumG], dt.float32,
                            kind="ExternalInput")
    attT = nc.dram_tensor("attT", [P, FU], dt.bfloat16, kind="ExternalInput")
    biasT = nc.dram_tensor("biasT", [P, FU], dt.float32, kind="ExternalInput")
    idT = nc.dram_tensor("idT", [P, P], dt.bfloat16, kind="ExternalInput")

    const = ctx.enter_context(tc.tile_pool(name="const", bufs=1))
    io = ctx.enter_context(tc.tile_pool(name="io", bufs=3))
    slabs = ctx.enter_context(tc.tile_pool(name="slabs", bufs=BUFS_SLAB))
    ttp = ctx.enter_context(tc.tile_pool(name="ttp", bufs=BUFS_SLAB + 1))
    psum = ctx.enter_context(tc.tile_pool(name="psum", bufs=4, space="PSUM"))
    small = ctx.enter_context(tc.tile_pool(name="small", bufs=BUFS_SMALL))
    scp = ctx.enter_context(tc.tile_pool(name="scp", bufs=2))

    att_s = const.tile([P, FU], dt.bfloat16)
    nc.sync.dma_start(att_s[:], attT[:, :])
    bias_s = const.tile([P, FU], dt.float32)
    nc.sync.dma_start(bias_s[:], biasT[:, :])
    id_s = const.tile([P, P], dt.bfloat16)
    nc.sync.dma_start(id_s[:], idT[:, :])
    al02 = const.tile([P, 1], dt.float32)
    nc.vector.memset(al02[:], 0.2)

    from concourse import library_config
    nc.gpsimd.load_library(library_config.standard)

    state = dict(att_s=att_s, bias_s=bias_s, id_s=id_s)

    def blocks():
        xr_w = mg_w = None
        for j in range(NB):
            g = int(G[j])
            o = int(off[j])
            if j % TBATCH == 0:
                nb = min(TBATCH, NB - j)
                rows = slice(j * P, (j + nb) * P)
                xr_w = io.tile([P, nb, FU], dt.bfloat16, tag="xr")
                nc.sync.dma_start(
                    xr_w[:], xr_d[rows, :].rearrange("(b d) f -> d b f", b=nb))
                gb = int(off[j + nb] - off[j])
                mg_w = io.tile([P, gb], dt.float32, tag="mg")
                nc.sync.dma_start(mg_w[:], mneg_d[:, o:o + gb])
                state["obatch"] = o
            xr_b = xr_w[:, j % TBATCH, :]

            ps = psum.tile([P, QSTEP * FM], dt.float32, tag="ps")
            qdone = 0
            first = True
            # chunk the strata so tile sizes stay bounded by QCH
            for qo in range(0, g, QCH):
                gc = min(QCH, g - qo)
                mg = mg_w[:, o + qo - state["obatch"]:
                          o + qo - state["obatch"] + gc]
                slab = slabs.tile([P, gc, FU], dt.bfloat16, tag="slab")
                nc.sync.dma_start(slab[:], slab_d[:, o + qo:o + qo + gc, :])

                tt = ttp.tile([P, gc, FU], dt.bfloat16, tag="tt")
                nc.vector.tensor_tensor(
                    tt[:], slab[:],
                    xr_b.unsqueeze(1).to_broadcast([P, gc, FU]), op.add)
                uu = slabs.tile([P, gc, FU], dt.bfloat16, tag="uu")
                if LRELU_MODE == "prelu":
                    nc.scalar.activation(uu[:], tt[:], AF.Prelu,
                                         alpha=al02[:])
                elif LRELU_MODE == "abs":
                    # lrelu(t) = 0.6t + 0.4|t| = 0.6*(t + |(2/3) t|); the
                    # 0.6 is folded into att host-side
                    aa = slabs.tile([P, gc, FU], dt.bfloat16, tag="aa")
                    nc.scalar.activation(aa[:], tt[:], AF.Abs, scale=2.0 / 3.0)
                    nc.vector.tensor_tensor(uu[:], tt[:], aa[:], op.add)
                else:
                    nc.vector.scalar_tensor_tensor(uu[:], tt[:], 0.2, tt[:],
                                                   op.mult, op.max)
                vv = ttp.tile([P, gc, FU], dt.bfloat16, tag="tt")
                nc.vector.tensor_tensor(
                    vv[:], uu[:],
                    att_s[:].unsqueeze(1).to_broadcast([P, gc, FU]), op.mult)

                # layer 2: two-step reduce - pair-add halves at 2x (bf16),
                # then 1x reduce over C/2; cuts DVE read volume ~1.6x.
                # (measured a small loss on layer 1 - keep single-step there)
                sc = small.tile([P, gc, H], dt.float32, tag="sc")
                if layer == 2:
                    ch = C // 2
                    vh = vv[:].rearrange("p g (h c) -> p g h c", c=C)
                    sc8 = scp.tile([P, gc, H, ch], dt.bfloat16, tag="sc8")
                    with nc.allow_low_precision(reason="pair bf16, tol 2e-2"):
                        nc.vector.tensor_tensor(sc8[:], vh[:, :, :, 0:ch],
                                                vh[:, :, :, ch:C], op.add)
                    nc.vector.tensor_reduce(sc[:], sc8[:],
                                            mybir.AxisListType.X, op.add)
                else:
                    nc.vector.tensor_reduce(
                        sc[:], vv[:].rearrange("p g (h c) -> p g h c", c=C),
                        mybir.AxisListType.X, op.add)
                sc2 = small.tile([P, gc, H], dt.float32, tag="sc2")
                nc.vector.tensor_tensor(
                    sc2[:], sc[:], mg.unsqueeze(2).to_broadcast([P, gc, H]),
                    op.add)

                ms = slabs.tile([P, gc, FM], dt.bfloat16, tag="ms")
                nc.scalar.activation(ms[:, :, FU:FM], sc2[:], AF.Exp)
                exv = ms[:, :, FU:FM]
                # split the exp-weighted multiply: low heads on DVE, high
                # heads on Pool, so the two run concurrently. For the DVE
                # share, Act writes exp() pre-expanded over C (packed in1)
                # so the DVE multiply runs in 2x mode.
                sp = min(MS_SPLIT * FU // F1, FU)
                hs_ = sp // C
                if sp > 0:
                    exe = scp.tile([P, gc, sp], dt.bfloat16, tag="exe")
                    nc.scalar.activation(
                        exe[:].rearrange("p g (h c) -> p g h c", c=C),
                        sc2[:, :, 0:hs_].unsqueeze(3).to_broadcast(
                            [P, gc, hs_, C]),
                        AF.Exp)
                    nc.vector.tensor_tensor(
                        ms[:, :, 0:sp], slab[:, :, 0:sp], exe[:], op.mult)
                if sp < FU:
                    nc.gpsimd.tensor_tensor(
                        ms[:, :, sp:FU].rearrange("p g (h c) -> p g h c", c=C),
                        slab[:, :, sp:FU].rearrange("p g (h c) -> p g h c",
                                                    c=C),
                        exv[:, :, hs_:H].unsqueeze(3).to_broadcast(
                            [P, gc, H - hs_, C]),
                        op.mult)

                q = 0
                while q + QSTEP <= gc:
                    nc.tensor.matmul(
                        ps[:], id_s[:],
                        ms[:, q:q + QSTEP, :].rearrange("p g f -> p (g f)"),
                        start=first, stop=(qdone + q + QSTEP == g))
                    first = False
                    q += QSTEP
                while q < gc:
                    nc.tensor.matmul(ps[:, 0:FM], id_s[:], ms[:, q, :],
                                     start=first, stop=(qdone + q == g - 1))
                    first = False
                    q += 1
                qdone += gc

            # strata 1..QSTEP-1 only hold data if at least one full group
            # ran; PSUM->SBUF copy on Act, strided DVE add of odd strata
            hsum = small.tile([P, FM], dt.float32, tag="hs")
            nc.vector.tensor_copy(hsum[:], ps[:, 0:FM])
            if g >= QSTEP and QSTEP >= 2:
                for s in range(1, QSTEP):
                    nc.vector.tensor_tensor(
                        hsum[:], hsum[:], ps[:, s * FM:(s + 1) * FM], op.add)

            dn = small.tile([P, H], dt.float32, tag="dn")
            nc.vector.tensor_scalar_add(dn[:], hsum[:, FU:FM], EPS)
            rd = small.tile([P, H], dt.float32, tag="rd")
            nc.vector.reciprocal(rd[:], dn[:])
            ov = small.tile([P, FU], dt.float32, tag="ov")
            nc.vector.tensor_tensor(
                ov[:].rearrange("p (h c) -> p h c", c=C),
                hsum[:, 0:FU].rearrange("p (h c) -> p h c", c=C),
                rd[:].unsqueeze(2).to_broadcast([P, H, C]),
                op.mult)
            ob = small.tile([P, FU], dt.float32, tag="ob")
            nc.vector.tensor_tensor(ob[:], ov[:], bias_s[:], op.add)

            yield j, g, ob

    return slab_d, state, blocks


def _build_e1(G, off, sumG):
    """Edge phase layer 1 + fused layer-2 node transform."""
    import concourse.mybir as mybir
    import concourse.tile as tile

    nc = _mk_bass()
    dt = mybir.dt
    op = mybir.AluOpType
    AF = mybir.ActivationFunctionType

    W2 = nc.dram_tensor("W2cat", [P, P], dt.bfloat16, kind="ExternalInput")
    B2 = nc.dram_tensor("B2col", [P, 1], dt.float32, kind="ExternalInput")
    xlr2T = nc.dram_tensor("xlr2T", [P, NOWN], dt.bfloat16,
                           kind="ExternalOutput")

    with tile.TileContext(nc) as tc, ExitStack() as ctx:
        _, state, blocks = _edge_pipeline(nc, tc, ctx, 1, G, off, sumG)
        work = ctx.enter_context(tc.tile_pool(name="t2", bufs=2))
        psum2 = ctx.enter_context(tc.tile_pool(name="psum2", bufs=2,
                                               space="PSUM"))

        w2_s = None
        b2_s = None
        out2 = None
        for j, g, ob in blocks():
            if w2_s is None:
                cpool = ctx.enter_context(tc.tile_pool(name="c2", bufs=1))
                w2_s = cpool.tile([P, P], dt.bfloat16)
                nc.sync.dma_start(w2_s[:], W2[:, :])
                b2_s = cpool.tile([P, 1], dt.float32)
                nc.sync.dma_start(b2_s[:], B2[:, :])
            # ELU -> h (bf16)
            mm = work.tile([P, F1], dt.float32, tag="mm")
            nc.vector.tensor_scalar_min(mm[:], ob[:], 0.0)
            em = work.tile([P, F1], dt.float32, tag="em")
            nc.scalar.activation(em[:], mm[:], AF.Exp)
            hf = work.tile([P, F1], dt.float32, tag="hf")
            nc.vector.scalar_tensor_tensor(hf[:], ob[:], 0.0, em[:],
                                           op.max, op.add)
            h16 = work.tile([P, F1], dt.bfloat16, tag="h16")
            nc.vector.tensor_scalar_add(h16[:], hf[:], -1.0)
            # layer-2 transform: hT then W2^T @ hT -> [fo, nodes]
            tp = psum2.tile([P, P], dt.bfloat16, tag="tp")
            nc.tensor.transpose(tp[:], h16[:], state["id_s"][:])
            hT = work.tile([P, P], dt.bfloat16, tag="hT")
            nc.vector.tensor_copy(hT[:], tp[:])
            p2 = psum2.tile([P, P], dt.float32, tag="p2")
            nc.tensor.matmul(p2[:], w2_s[:], hT[:], start=True, stop=True)
            if j % TBATCH == 0:
                out2 = work.tile([P, min(TBATCH, NB - j), P], dt.bfloat16,
                                 tag="out2")
            nc.vector.tensor_scalar_add(out2[:, j % TBATCH, :], p2[:],
                                        b2_s[:, 0:1])
            if j % TBATCH == min(TBATCH, NB - (j // TBATCH) * TBATCH) - 1 \
                    or j == NB - 1:
                jj = (j // TBATCH) * TBATCH
                nb = j - jj + 1
                nc.sync.dma_start(xlr2T[:, jj * P:(jj + nb) * P],
                                  out2[:, 0:nb, :])
    nc.compile()
    return nc


def _build_e2(G, off, sumG):
    """Edge phase layer 2 + log-softmax tail."""
    import concourse.mybir as mybir
    import concourse.tile as tile

    nc = _mk_bass()
    dt = mybir.dt
    op = mybir.AluOpType
    AF = mybir.ActivationFunctionType

    out_d = nc.dram_tensor("out", [NOWN, FOUT], dt.float32,
                           kind="ExternalOutput")

    with tile.TileContext(nc) as tc, ExitStack() as ctx:
        _, state, blocks = _edge_pipeline(nc, tc, ctx, 2, G, off, sumG)
        persist = ctx.enter_context(tc.tile_pool(name="persist", bufs=1))
        work = ctx.enter_context(tc.tile_pool(name="ls", bufs=2))

        mx_all = persist.tile([P, NB], dt.float32)
        s_all = persist.tile([P, NB], dt.float32)
        y_tiles = []
        for j, g, ob in blocks():
            yb = persist.tile([P, F2P], dt.float32, tag=f"y{j}", name=f"y{j}")
            nc.vector.tensor_copy(yb[:], ob[:])
            yr = yb[:].rearrange("p (h c) -> p h c", c=C2)[:, :, 0:DOUT]
            mx2 = mx_all[:, j:j + 1]
            nc.vector.tensor_reduce(mx2, yr, mybir.AxisListType.XY, op.max)
            mxn = work.tile([P, 1], dt.float32, tag="mxn")
            nc.vector.tensor_scalar_mul(mxn[:], mx2, -1.0)
            et = work.tile([P, FOUT], dt.float32, tag="et")
            nc.scalar.activation(
                et[:].rearrange("p (h c) -> p h c", c=DOUT), yr,
                AF.Exp, bias=mxn[:])
            nc.vector.tensor_reduce(s_all[:, j:j + 1], et[:],
                                    mybir.AxisListType.X, op.add)
            y_tiles.append(yb)

        # ln(S) via exponent/mantissa split (no Ln in the loaded act table):
        # ln(S) = (e - 127)*ln2 + poly(m), m in [1, 2)
        C5, C4, C3, C2_, C1, C0 = (0.030102625011658456,
                                   -0.2806325404494927,
                                   1.1048082361987304,
                                   -2.4208125632180866,
                                   3.4982279012091095,
                                   -1.9316715417207186)
        bits = s_all[:].bitcast(dt.int32)
        ei = persist.tile([P, NB], dt.int32)
        nc.vector.tensor_scalar(ei[:], bits, 23, None, op.arith_shift_right)
        ef = persist.tile([P, NB], dt.float32)
        nc.vector.tensor_copy(ef[:], ei[:])
        mi = persist.tile([P, NB], dt.int32)
        nc.vector.tensor_scalar(mi[:], bits, 0x007FFFFF, 0x3F800000,
                                op.bitwise_and, op.bitwise_or)
        mf = mi[:].bitcast(dt.float32)
        pp = persist.tile([P, NB], dt.float32)
        nc.vector.tensor_scalar(pp[:], mf, C5, C4, op.mult, op.add)
        qq = persist.tile([P, NB], dt.float32)
        for ck in (C3, C2_, C1, C0):
            nc.vector.tensor_tensor(qq[:], pp[:], mf, op.mult)
            nc.vector.tensor_scalar_add(pp[:], qq[:], ck)
        ct_all = persist.tile([P, NB], dt.float32)
        nc.vector.scalar_tensor_tensor(
            ct_all[:], ef[:], 0.6931471805599453, pp[:], op.mult, op.add)
        ct2 = persist.tile([P, NB], dt.float32)
        nc.vector.scalar_tensor_tensor(
            ct2[:], ct_all[:], -127.0 * 0.6931471805599453, mx_all[:],
            op.add, op.add)
        orow = 0
        for j in range(NB):
            yr = y_tiles[j][:].rearrange("p (h c) -> p h c",
                                         c=C2)[:, :, 0:DOUT]
            of = work.tile([P, FOUT], dt.float32, tag="of")
            nc.vector.tensor_scalar_sub(
                of[:].rearrange("p (h c) -> p h c", c=DOUT), yr,
                ct2[:, j:j + 1])
            nc.sync.dma_start(out_d[orow:orow + P, :], of[:])
            orow += P
    nc.compile()
    return nc


# ---------------------------------------------------------------------------
# runner
# ---------------------------------------------------------------------------

_state = {}


def _run(nc, in_maps, trace=False):
    from concourse.bass_utils import run_bass_kernel_spmd
    return run_bass_kernel_spmd(nc, in_maps, core_ids=list(range(NCORES)),
                                trace=trace)


def _bcast_rows(v, rows=P):
    return np.ascontiguousarray(np.broadcast_to(np.asarray(v)[None, :],
                                                (rows, len(v))))


def kernel(x, edge_index, Wl1, bl1, Wr1, br1, att1, bias1,
           Wl2, bl2, Wr2, br2, att2, bias2, _trace=False, _times=None):
    x = np.asarray(x, _f32)
    edge_index = np.asarray(edge_index)

    g = _prep_graph(edge_index)
    members, G, off, sumG = g["members"], g["G"], g["off"], g["sumG"]

    ckey = tuple(G)
    if _state.get("ckey") != ckey:
        _state["ckey"] = ckey
        _state["nc_t1"] = _build_t1()
        _state["nc_e1"] = _build_e1(G, off, sumG)
        _state["nc_e2"] = _build_e2(G, off, sumG)

    id128 = np.eye(P, dtype=_bf16)

    def gather_nodes(arr, mem):
        flat = mem.reshape(-1)
        out = arr[np.maximum(flat, 0)]
        out[flat < 0] = 0
        return out

    def trace_run(key, nc, in_maps):
        r = _run(nc, in_maps, trace=_trace)
        if _times is not None:
            _times[key] = r.exec_time_ns
            if r.instructions_and_trace is not None:
                _times["_" + key + "_insts"] = r.instructions_and_trace
        return r.results

    # ---- T1 ----
    W1 = np.concatenate([Wl1, Wr1], axis=1).astype(_bf16)      # [128, 256]
    B1t = _bcast_rows(np.concatenate([bl1, br1]).astype(_f32))
    t1_maps = []
    for k in range(NCORES):
        xg = gather_nodes(x, members[k]).astype(_bf16)         # [6272, 128]
        t1_maps.append({"xT": np.ascontiguousarray(xg.T),
                        "Wcat": W1, "Bcat": B1t})
    r1 = trace_run("t1", _state["nc_t1"], t1_maps)

    # assemble xl by node id, then expand per-edge slabs (data movement)
    xl_byid = np.zeros((N, F1), _bf16)
    for k in range(NCORES):
        flat = members[k].reshape(-1)
        ok = flat >= 0
        xl_byid[flat[ok]] = r1[k]["xl"][ok]

    ascale = 0.6 if LRELU_MODE == "abs" else 1.0
    att1_t = _bcast_rows(ascale * np.asarray(att1, _f32).reshape(-1)).astype(_bf16)
    bias1_t = _bcast_rows(bias1).astype(_f32)
    Wl2p = np.zeros((P, F2P), _f32)
    Wl2p.reshape(P, H, C2)[:, :, :DOUT] = np.asarray(Wl2, _f32).reshape(P, H, DOUT)
    Wr2p = np.zeros((P, F2P), _f32)
    Wr2p.reshape(P, H, C2)[:, :, :DOUT] = np.asarray(Wr2, _f32).reshape(P, H, DOUT)
    W2cat = np.ascontiguousarray(
        np.concatenate([Wl2p, Wr2p], axis=1)).astype(_bf16)    # [128,128]
    b2 = np.zeros(P, _f32)
    b2.reshape(2, H, C2)[0, :, :DOUT] = np.asarray(bl2, _f32).reshape(H, DOUT)
    b2.reshape(2, H, C2)[1, :, :DOUT] = np.asarray(br2, _f32).reshape(H, DOUT)
    B2col = np.ascontiguousarray(b2[:, None])

    e1_maps = []
    for k in range(NCORES):
        slab1 = xl_byid[g["idx"][k]]                           # [P, sumG, 128]
        e1_maps.append({"slab": slab1, "xr": r1[k]["xr"],
                        "mneg": g["mneg"][k],
                        "attT": att1_t, "biasT": bias1_t, "idT": id128,
                        "W2cat": W2cat, "B2col": B2col})
    re1 = trace_run("e1", _state["nc_e1"], e1_maps)

    # split xlr2T into xl2 (by node id) and xr2 rows (data movement)
    xl2_byid = np.zeros((N, F2P), _bf16)
    xr2 = [None] * NCORES
    for k in range(NCORES):
        lr = re1[k]["xlr2T"]                                   # [128, NOWN]
        flat = members[k].reshape(-1)
        ok = flat >= 0
        xl2_byid[flat[ok]] = lr[0:F2P].T[ok]
        xr2[k] = np.ascontiguousarray(lr[F2P:P].T)             # [NOWN, 64]

    att2p = np.zeros((H, C2), _f32)
    att2p[:, :DOUT] = ascale * np.asarray(att2, _f32)
    att2_t = _bcast_rows(att2p.reshape(-1)).astype(_bf16)      # [128, 64]
    bias2p = np.zeros(F2P, _f32)
    bias2p.reshape(H, C2)[:, :DOUT] = np.asarray(bias2, _f32).reshape(H, DOUT)
    bias2_t = _bcast_rows(bias2p)

    e2_maps = []
    for k in range(NCORES):
        slab2 = xl2_byid[g["idx"][k]]                          # [P, sumG, 64]
        e2_maps.append({"slab": slab2, "xr": xr2[k],
                        "mneg": g["mneg"][k],
                        "attT": att2_t, "biasT": bias2_t, "idT": id128})
    re2 = trace_run("e2", _state["nc_e2"], e2_maps)

    out = np.zeros((N, FOUT), _f32)
    for k in range(NCORES):
        flat = members[k].reshape(-1)
        ok = flat >= 0
        out[flat[ok]] = re2[k]["out"][ok]
    return out


# revision 43
# speedup vs baseline: 1.0936x; 1.0936x over previous
"""GATv2 (2-layer, 8-head) Trainium2 kernel, 8-core node-sharded.

v2 design (host-expanded slabs, no device-side gather):

  T1 NEFF:   per-core node transform xl/xr = x@W{l,r}+b via bf16 matmuls.
  host:      assembles xl by node id, expands the per-edge slab
             slab[d, off_j+q, :] = xl[src of q-th in-edge of node at block
             slot (j, d)] (pure data movement), so the edge NEFF streams it
             densely at full HBM bandwidth instead of paying ~10ns/row of
             GPSIMD descriptor-generation ucode for a gathering DMA.
  E1 NEFF:   per-edge score (DVE add -> Act leaky-relu -> DVE mult ->
             DVE segmented reduce), segment softmax without max-subtraction
             (scores are O(10), fp32 exp is safe), exp-weighted aggregation
             via paired identity matmuls accumulating in PSUM, ELU, plus the
             fused layer-2 node transform (PE transpose + matmul) emitting
             xl2/xr2 as a [128, NOWN] feature-major tensor.
  host:      expands slab2 from xl2 rows, transposes xr2.
  E2 NEFF:   same edge pipeline at F=64 with quad identity matmuls and the
             log-softmax tail (ln via exponent/mantissa polynomial).

Edges are laid out stratum-major: edge slot (q, d) of block j holds the
q-th in-edge of the node at partition d, so the xr broadcast is a plain
broadcast AP, and segment max/sum are free-dim reduces. All cores share the
per-slot stratum counts G[j] so a single NEFF serves all 8 cores (SPMD).
"""

import os
from contextlib import ExitStack

import ml_dtypes
import numpy as np

N, E0, DIN, H, DH, DOUT = 50000, 1600000, 128, 8, 16, 7
F1 = H * DH            # 128
C2 = 8                 # layer-2 per-head cols in slab (7 real + 1 pad)
F2P = H * C2           # 64
NCORES = 8
P = 128
NBLK = 392             # 392*128 = 50176 >= N, 392 % 8 == 0
NB = NBLK // NCORES    # 49 blocks per core
NOWN = NB * P          # 6272 nodes per core (incl. pad slots)
NPAD = NBLK * P        # 50176
FM1 = F1 + H           # 136 (agg | denom)
FM2 = F2P + H          # 72
FOUT = H * DOUT        # 56
NEG = -1.0e9
EPS = 1e-16
TBATCH = 8             # blocks per batched DMA

_f32 = np.float32
_bf16 = ml_dtypes.bfloat16

# tuning switches (validated on hardware)
LRELU_MODE = "prelu"   # "prelu" (Act) | "abs" (Act Abs + DVE add) | "stt" (DVE)
MS_SPLIT = 80          # feature cols 0:MS_SPLIT of Ms on DVE, rest on Pool
SC_BF16 = False        # bf16-out reduce is broken on HW (measured) - keep f32
BUFS_SLAB = 3          # pipeline depth for slab-sized tiles
BUFS_SMALL = 3


# ---------------------------------------------------------------------------
# host-side graph preprocessing (pure index/layout manipulation)
# ---------------------------------------------------------------------------

def _prep_graph(edge_index):
    src = np.concatenate([edge_index[0], np.arange(N, dtype=np.int64)])
    dst = np.concatenate([edge_index[1], np.arange(N, dtype=np.int64)])
    deg = np.bincount(dst, minlength=N).astype(np.int64)

    # group nodes into blocks of 128 with near-equal degree
    order = np.argsort(deg, kind="stable")
    nodes_sorted = np.concatenate([order, np.full(NPAD - N, -1, np.int64)])
    blocks = nodes_sorted.reshape(NBLK, P)          # [392, 128]
    blkmax = np.where(blocks >= 0, deg[np.maximum(blocks, 0)], 0).max(axis=1)
    G = blkmax.reshape(NB, NCORES).max(axis=1).astype(int)   # [49] shared
    G = np.maximum(G, 1)
    off = np.concatenate([[0], np.cumsum(G)]).astype(int)
    sumG = int(off[-1])

    # per-node padded src lists
    oe = np.argsort(dst, kind="stable")
    ss = dst[oe]
    starts = np.searchsorted(ss, np.arange(N))
    pos = np.arange(len(ss)) - starts[ss]
    Amax = max(int(deg.max()), int(G.max()))
    pad = np.zeros((N, Amax), np.int32)
    pad[ss, pos] = src[oe]

    members = [None] * NCORES
    idx = [None] * NCORES
    mneg = [None] * NCORES
    for k in range(NCORES):
        mem = blocks[np.arange(NB) * NCORES + k]   # [49, 128]
        members[k] = mem
        ia = np.zeros((P, sumG), np.int32)
        mg = np.full((P, sumG), NEG, _f32)
        for j in range(NB):
            g = G[j]
            m = mem[j]
            msafe = np.maximum(m, 0)
            darr = np.where(m >= 0, deg[msafe], 0)
            ia[:, off[j]:off[j] + g] = pad[msafe, :g]
            mg[:, off[j]:off[j] + g] = np.where(
                np.arange(g)[None, :] < darr[:, None], 0.0, NEG)
        idx[k], mneg[k] = ia, mg

    return dict(members=members, G=G, off=off, sumG=sumG, idx=idx, mneg=mneg)


# ---------------------------------------------------------------------------
# NEFF builders
# ---------------------------------------------------------------------------

def _mk_bass():
    import concourse.bacc as bacc
    return bacc.Bacc("TRN2", target_bir_lowering=False)


def _build_t1():
    """xT [128, NOWN] bf16 @ Wcat [128, 256] -> xl rows + xr rows (bf16)."""
    import concourse.mybir as mybir
    import concourse.tile as tile

    nc = _mk_bass()
    dt = mybir.dt
    op = mybir.AluOpType
    fo = 2 * F1

    xT = nc.dram_tensor("xT", [P, NOWN], dt.bfloat16, kind="ExternalInput")
    W = nc.dram_tensor("Wcat", [P, fo], dt.bfloat16, kind="ExternalInput")
    B = nc.dram_tensor("Bcat", [P, fo], dt.float32, kind="ExternalInput")
    xl = nc.dram_tensor("xl", [NOWN, F1], dt.bfloat16, kind="ExternalOutput")
    xr = nc.dram_tensor("xr", [NOWN, F1], dt.bfloat16, kind="ExternalOutput")

    with tile.TileContext(nc) as tc, ExitStack() as ctx:
        const = ctx.enter_context(tc.tile_pool(name="const", bufs=1))
        work = ctx.enter_context(tc.tile_pool(name="work", bufs=2))
        psum = ctx.enter_context(tc.tile_pool(name="psum", bufs=3, space="PSUM"))

        w_s = const.tile([P, fo], dt.bfloat16)
        nc.sync.dma_start(w_s[:], W[:, :])
        b_s = const.tile([P, fo], dt.float32)
        nc.sync.dma_start(b_s[:], B[:, :])

        for jj in range(0, NB, TBATCH):
            nb = min(TBATCH, NB - jj)
            lhs = work.tile([P, nb, P], dt.bfloat16, tag="lhs")
            nc.sync.dma_start(lhs[:], xT[:, jj * P:(jj + nb) * P])
            ol = work.tile([P, nb, F1], dt.bfloat16, tag="ol")
            orr = work.tile([P, nb, F1], dt.bfloat16, tag="orr")
            for b in range(nb):
                ps = psum.tile([P, fo], dt.float32, tag="ps")
                nc.tensor.matmul(ps[:], lhs[:, b, :], w_s[:],
                                 start=True, stop=True)
                nc.vector.tensor_tensor(ol[:, b, :], ps[:, 0:F1],
                                        b_s[:, 0:F1], op.add)
                nc.vector.tensor_tensor(orr[:, b, :], ps[:, F1:fo],
                                        b_s[:, F1:fo], op.add)
            rows = slice(jj * P, (jj + nb) * P)
            nc.sync.dma_start(
                xl[rows, :].rearrange("(b d) f -> d b f", b=nb), ol[:])
            nc.sync.dma_start(
                xr[rows, :].rearrange("(b d) f -> d b f", b=nb), orr[:])
    nc.compile()
    return nc


def _edge_pipeline(nc, tc, ctx, layer, G, off, sumG):
    """Shared edge phase. Returns per-layer specifics handled by caller."""
    import concourse.mybir as mybir

    dt = mybir.dt
    op = mybir.AluOpType
    AF = mybir.ActivationFunctionType

    FU = F1 if layer == 1 else F2P
    C = DH if layer == 1 else C2
    FM = FU + H
    QSTEP = 2 if layer == 1 else 4   # matmul batch; QSTEP*FM*4B <= 2KB bank
    QCH = 48                         # edge strata per chunk (bounds tiles)

    slab_d = nc.dram_tensor("slab", [P, sumG, FU], dt.bfloat16,
                            kind="ExternalInput")
    xr_d = nc.dram_tensor("xr", [NOWN, FU], dt.bfloat16, kind="ExternalInput")
    mneg_d = nc.dram_tensor("mneg", [P, sumG], dt.float32,
                            kind="ExternalInput")
    attT = nc.dram_tensor("attT", [P, FU], dt.bfloat16, kind="ExternalInput")
    biasT = nc.dram_tensor("biasT", [P, FU], dt.float32, kind="ExternalInput")
    idT = nc.dram_tensor("idT", [P, P], dt.bfloat16, kind="ExternalInput")

    # e2's tiles are half-width, leaving ~90KB SBUF headroom: deepen its
    # pipeline rings (the edge NEFFs are latency-bound, not throughput-bound)
    xd = 0 if layer == 1 else 2
    const = ctx.enter_context(tc.tile_pool(name="const", bufs=1))
    io = ctx.enter_context(tc.tile_pool(name="io", bufs=3 + (xd > 0)))
    slabs = ctx.enter_context(tc.tile_pool(name="slabs", bufs=BUFS_SLAB + xd))
    ttp = ctx.enter_context(tc.tile_pool(name="ttp",
                                         bufs=BUFS_SLAB + 1 + xd))
    psum = ctx.enter_context(tc.tile_pool(name="psum", bufs=4 + xd,
                                          space="PSUM"))
    small = ctx.enter_context(tc.tile_pool(name="small",
                                           bufs=BUFS_SMALL + xd))
    scp = ctx.enter_context(tc.tile_pool(name="scp", bufs=2 + xd))

    att_s = const.tile([P, FU], dt.bfloat16)
    nc.sync.dma_start(att_s[:], attT[:, :])
    bias_s = const.tile([P, FU], dt.float32)
    nc.sync.dma_start(bias_s[:], biasT[:, :])
    id_s = const.tile([P, P], dt.bfloat16)
    nc.sync.dma_start(id_s[:], idT[:, :])
    al02 = const.tile([P, 1], dt.float32)
    nc.vector.memset(al02[:], 0.2)

    from concourse import library_config
    nc.gpsimd.load_library(library_config.standard)

    state = dict(att_s=att_s, bias_s=bias_s, id_s=id_s)

    def blocks():
        xr_w = mg_w = None
        for j in range(NB):
            g = int(G[j])
            o = int(off[j])
            if j % TBATCH == 0:
                nb = min(TBATCH, NB - j)
                rows = slice(j * P, (j + nb) * P)
                xr_w = io.tile([P, nb, FU], dt.bfloat16, tag="xr")
                nc.sync.dma_start(
                    xr_w[:], xr_d[rows, :].rearrange("(b d) f -> d b f", b=nb))
                gb = int(off[j + nb] - off[j])
                mg_w = io.tile([P, gb], dt.float32, tag="mg")
                nc.sync.dma_start(mg_w[:], mneg_d[:, o:o + gb])
                state["obatch"] = o
            xr_b = xr_w[:, j % TBATCH, :]

            ps = psum.tile([P, QSTEP * FM], dt.float32, tag="ps")
            qdone = 0
            first = True
            # chunk the strata so tile sizes stay bounded by QCH
            for qo in range(0, g, QCH):
                gc = min(QCH, g - qo)
                mg = mg_w[:, o + qo - state["obatch"]:
                          o + qo - state["obatch"] + gc]
                slab = slabs.tile([P, gc, FU], dt.bfloat16, tag="slab")
                nc.sync.dma_start(slab[:], slab_d[:, o + qo:o + qo + gc, :])

                tt = ttp.tile([P, gc, FU], dt.bfloat16, tag="tt")
                nc.vector.tensor_tensor(
                    tt[:], slab[:],
                    xr_b.unsqueeze(1).to_broadcast([P, gc, FU]), op.add)
                uu = slabs.tile([P, gc, FU], dt.bfloat16, tag="uu")
                if LRELU_MODE == "prelu":
                    nc.scalar.activation(uu[:], tt[:], AF.Prelu,
                                         alpha=al02[:])
                elif LRELU_MODE == "abs":
                    # lrelu(t) = 0.6t + 0.4|t| = 0.6*(t + |(2/3) t|); the
                    # 0.6 is folded into att host-side
                    aa = slabs.tile([P, gc, FU], dt.bfloat16, tag="aa")
                    nc.scalar.activation(aa[:], tt[:], AF.Abs, scale=2.0 / 3.0)
                    nc.vector.tensor_tensor(uu[:], tt[:], aa[:], op.add)
                else:
                    nc.vector.scalar_tensor_tensor(uu[:], tt[:], 0.2, tt[:],
                                                   op.mult, op.max)
                vv = ttp.tile([P, gc, FU], dt.bfloat16, tag="tt")
                nc.vector.tensor_tensor(
                    vv[:], uu[:],
                    att_s[:].unsqueeze(1).to_broadcast([P, gc, FU]), op.mult)

                # layer 2: two-step reduce - pair-add halves at 2x (bf16),
                # then 1x reduce over C/2; cuts DVE read volume ~1.6x.
                # (measured a small loss on layer 1 - keep single-step there)
                sc = small.tile([P, gc, H], dt.float32, tag="sc")
                if layer == 2:
                    ch = C // 2
                    vh = vv[:].rearrange("p g (h c) -> p g h c", c=C)
                    sc8 = scp.tile([P, gc, H, ch], dt.bfloat16, tag="sc8")
                    with nc.allow_low_precision(reason="pair bf16, tol 2e-2"):
                        nc.vector.tensor_tensor(sc8[:], vh[:, :, :, 0:ch],
                                                vh[:, :, :, ch:C], op.add)
                    nc.vector.tensor_reduce(sc[:], sc8[:],
                                            mybir.AxisListType.X, op.add)
                else:
                    nc.vector.tensor_reduce(
                        sc[:], vv[:].rearrange("p g (h c) -> p g h c", c=C),
                        mybir.AxisListType.X, op.add)
                sc2 = small.tile([P, gc, H], dt.float32, tag="sc2")
                nc.vector.tensor_tensor(
                    sc2[:], sc[:], mg.unsqueeze(2).to_broadcast([P, gc, H]),
                    op.add)

                ms = slabs.tile([P, gc, FM], dt.bfloat16, tag="ms")
                nc.scalar.activation(ms[:, :, FU:FM], sc2[:], AF.Exp)
                exv = ms[:, :, FU:FM]
                # split the exp-weighted multiply: low heads on DVE, high
                # heads on Pool, so the two run concurrently
                sp = min(MS_SPLIT * FU // F1, FU)
                hs_ = sp // C
                if sp > 0:
                    nc.vector.tensor_tensor(
                        ms[:, :, 0:sp].rearrange("p g (h c) -> p g h c", c=C),
                        slab[:, :, 0:sp].rearrange("p g (h c) -> p g h c",
                                                   c=C),
                        exv[:, :, 0:hs_].unsqueeze(3).to_broadcast(
                            [P, gc, hs_, C]),
                        op.mult)
                if sp < FU:
                    nc.gpsimd.tensor_tensor(
                        ms[:, :, sp:FU].rearrange("p g (h c) -> p g h c", c=C),
                        slab[:, :, sp:FU].rearrange("p g (h c) -> p g h c",
                                                    c=C),
                        exv[:, :, hs_:H].unsqueeze(3).to_broadcast(
                            [P, gc, H - hs_, C]),
                        op.mult)

                q = 0
                while q + QSTEP <= gc:
                    nc.tensor.matmul(
                        ps[:], id_s[:],
                        ms[:, q:q + QSTEP, :].rearrange("p g f -> p (g f)"),
                        start=first, stop=(qdone + q + QSTEP == g))
                    first = False
                    q += QSTEP
                while q < gc:
                    nc.tensor.matmul(ps[:, 0:FM], id_s[:], ms[:, q, :],
                                     start=first, stop=(qdone + q == g - 1))
                    first = False
                    q += 1
                qdone += gc

            # strata 1..QSTEP-1 only hold data if at least one full group
            # ran; PSUM->SBUF copy on Act, strided DVE add of odd strata
            hsum = small.tile([P, FM], dt.float32, tag="hs")
            nc.vector.tensor_copy(hsum[:], ps[:, 0:FM])
            if g >= QSTEP and QSTEP >= 2:
                for s in range(1, QSTEP):
                    nc.vector.tensor_tensor(
                        hsum[:], hsum[:], ps[:, s * FM:(s + 1) * FM], op.add)

            dn = small.tile([P, H], dt.float32, tag="dn")
            nc.vector.tensor_scalar_add(dn[:], hsum[:, FU:FM], EPS)
            rd = small.tile([P, H], dt.float32, tag="rd")
            nc.vector.reciprocal(rd[:], dn[:])
            ov = small.tile([P, FU], dt.float32, tag="ov")
            nc.vector.tensor_tensor(
                ov[:].rearrange("p (h c) -> p h c", c=C),
                hsum[:, 0:FU].rearrange("p (h c) -> p h c", c=C),
                rd[:].unsqueeze(2).to_broadcast([P, H, C]),
                op.mult)
            ob = small.tile([P, FU], dt.float32, tag="ob")
            nc.vector.tensor_tensor(ob[:], ov[:], bias_s[:], op.add)

            yield j, g, ob

    return slab_d, state, blocks


def _build_e1(G, off, sumG):
    """Edge phase layer 1 + fused layer-2 node transform."""
    import concourse.mybir as mybir
    import concourse.tile as tile

    nc = _mk_bass()
    dt = mybir.dt
    op = mybir.AluOpType
    AF = mybir.ActivationFunctionType

    W2 = nc.dram_tensor("W2cat", [P, P], dt.bfloat16, kind="ExternalInput")
    B2 = nc.dram_tensor("B2col", [P, 1], dt.float32, kind="ExternalInput")
    xlr2T = nc.dram_tensor("xlr2T", [P, NOWN], dt.bfloat16,
                           kind="ExternalOutput")

    with tile.TileContext(nc) as tc, ExitStack() as ctx:
        _, state, blocks = _edge_pipeline(nc, tc, ctx, 1, G, off, sumG)
        work = ctx.enter_context(tc.tile_pool(name="t2", bufs=2))
        psum2 = ctx.enter_context(tc.tile_pool(name="psum2", bufs=2,
                                               space="PSUM"))

        w2_s = None
        b2_s = None
        out2 = None
        for j, g, ob in blocks():
            if w2_s is None:
                cpool = ctx.enter_context(tc.tile_pool(name="c2", bufs=1))
                w2_s = cpool.tile([P, P], dt.bfloat16)
                nc.sync.dma_start(w2_s[:], W2[:, :])
                b2_s = cpool.tile([P, 1], dt.float32)
                nc.sync.dma_start(b2_s[:], B2[:, :])
            # ELU -> h (bf16)
            mm = work.tile([P, F1], dt.float32, tag="mm")
            nc.vector.tensor_scalar_min(mm[:], ob[:], 0.0)
            em = work.tile([P, F1], dt.float32, tag="em")
            nc.scalar.activation(em[:], mm[:], AF.Exp)
            hf = work.tile([P, F1], dt.float32, tag="hf")
            nc.vector.scalar_tensor_tensor(hf[:], ob[:], 0.0, em[:],
                                           op.max, op.add)
            h16 = work.tile([P, F1], dt.bfloat16, tag="h16")
            nc.vector.tensor_scalar_add(h16[:], hf[:], -1.0)
            # layer-2 transform: hT then W2^T @ hT -> [fo, nodes]
            tp = psum2.tile([P, P], dt.bfloat16, tag="tp")
            nc.tensor.transpose(tp[:], h16[:], state["id_s"][:])
            hT = work.tile([P, P], dt.bfloat16, tag="hT")
            nc.vector.tensor_copy(hT[:], tp[:])
            p2 = psum2.tile([P, P], dt.float32, tag="p2")
            nc.tensor.matmul(p2[:], w2_s[:], hT[:], start=True, stop=True)
            if j % TBATCH == 0:
                out2 = work.tile([P, min(TBATCH, NB - j), P], dt.bfloat16,
                                 tag="out2")
            nc.vector.tensor_scalar_add(out2[:, j % TBATCH, :], p2[:],
                                        b2_s[:, 0:1])
            if j % TBATCH == min(TBATCH, NB - (j // TBATCH) * TBATCH) - 1 \
                    or j == NB - 1:
                jj = (j // TBATCH) * TBATCH
                nb = j - jj + 1
                nc.sync.dma_start(xlr2T[:, jj * P:(jj + nb) * P],
                                  out2[:, 0:nb, :])
    nc.compile()
    return nc


def _build_e2(G, off, sumG):
    """Edge phase layer 2 + log-softmax tail."""
    import concourse.mybir as mybir
    import concourse.tile as tile

    nc = _mk_bass()
    dt = mybir.dt
    op = mybir.AluOpType
    AF = mybir.ActivationFunctionType

    out_d = nc.dram_tensor("out", [NOWN, FOUT], dt.float32,
                           kind="ExternalOutput")

    with tile.TileContext(nc) as tc, ExitStack() as ctx:
        _, state, blocks = _edge_pipeline(nc, tc, ctx, 2, G, off, sumG)
        persist = ctx.enter_context(tc.tile_pool(name="persist", bufs=1))
        work = ctx.enter_context(tc.tile_pool(name="ls", bufs=2))

        mx_all = persist.tile([P, NB], dt.float32)
        s_all = persist.tile([P, NB], dt.float32)
        y_tiles = []
        for j, g, ob in blocks():
            yb = persist.tile([P, F2P], dt.float32, tag=f"y{j}", name=f"y{j}")
            nc.vector.tensor_copy(yb[:], ob[:])
            yr = yb[:].rearrange("p (h c) -> p h c", c=C2)[:, :, 0:DOUT]
            mx2 = mx_all[:, j:j + 1]
            nc.vector.tensor_reduce(mx2, yr, mybir.AxisListType.XY, op.max)
            mxn = work.tile([P, 1], dt.float32, tag="mxn")
            nc.vector.tensor_scalar_mul(mxn[:], mx2, -1.0)
            et = work.tile([P, FOUT], dt.float32, tag="et")
            nc.scalar.activation(
                et[:].rearrange("p (h c) -> p h c", c=DOUT), yr,
                AF.Exp, bias=mxn[:])
            nc.vector.tensor_reduce(s_all[:, j:j + 1], et[:],
                                    mybir.AxisListType.X, op.add)
            y_tiles.append(yb)

        # ln(S) via exponent/mantissa split (no Ln in the loaded act table):
        # ln(S) = (e - 127)*ln2 + poly(m), m in [1, 2)
        C5, C4, C3, C2_, C1, C0 = (0.030102625011658456,
                                   -0.2806325404494927,
                                   1.1048082361987304,
                                   -2.4208125632180866,
                                   3.4982279012091095,
                                   -1.9316715417207186)
        bits = s_all[:].bitcast(dt.int32)
        ei = persist.tile([P, NB], dt.int32)
        nc.vector.tensor_scalar(ei[:], bits, 23, None, op.arith_shift_right)
        ef = persist.tile([P, NB], dt.float32)
        nc.vector.tensor_copy(ef[:], ei[:])
        mi = persist.tile([P, NB], dt.int32)
        nc.vector.tensor_scalar(mi[:], bits, 0x007FFFFF, 0x3F800000,
                                op.bitwise_and, op.bitwise_or)
        mf = mi[:].bitcast(dt.float32)
        pp = persist.tile([P, NB], dt.float32)
        nc.vector.tensor_scalar(pp[:], mf, C5, C4, op.mult, op.add)
        qq = persist.tile([P, NB], dt.float32)
        for ck in (C3, C2_, C1, C0):
            nc.vector.tensor_tensor(qq[:], pp[:], mf, op.mult)
            nc.vector.tensor_scalar_add(pp[:], qq[:], ck)
        ct_all = persist.tile([P, NB], dt.float32)
        nc.vector.scalar_tensor_tensor(
            ct_all[:], ef[:], 0.6931471805599453, pp[:], op.mult, op.add)
        ct2 = persist.tile([P, NB], dt.float32)
        nc.vector.scalar_tensor_tensor(
            ct2[:], ct_all[:], -127.0 * 0.6931471805599453, mx_all[:],
            op.add, op.add)
        orow = 0
        for j in range(NB):
            yr = y_tiles[j][:].rearrange("p (h c) -> p h c",
                                         c=C2)[:, :, 0:DOUT]
            of = work.tile([P, FOUT], dt.float32, tag="of")
            nc.vector.tensor_scalar_sub(
                of[:].rearrange("p (h c) -> p h c", c=DOUT), yr,
                ct2[:, j:j + 1])
            nc.sync.dma_start(out_d[orow:orow + P, :], of[:])
            orow += P
    nc.compile()
    return nc


# ---------------------------------------------------------------------------
# runner
# ---------------------------------------------------------------------------

_state = {}


def _run(nc, in_maps, trace=False):
    from concourse.bass_utils import run_bass_kernel_spmd
    return run_bass_kernel_spmd(nc, in_maps, core_ids=list(range(NCORES)),
                                trace=trace)


def _bcast_rows(v, rows=P):
    return np.ascontiguousarray(np.broadcast_to(np.asarray(v)[None, :],
                                                (rows, len(v))))


def kernel(x, edge_index, Wl1, bl1, Wr1, br1, att1, bias1,
           Wl2, bl2, Wr2, br2, att2, bias2, _trace=False, _times=None):
    x = np.asarray(x, _f32)
    edge_index = np.asarray(edge_index)

    g = _prep_graph(edge_index)
    members, G, off, sumG = g["members"], g["G"], g["off"], g["sumG"]

    ckey = tuple(G)
    if _state.get("ckey") != ckey:
        _state["ckey"] = ckey
        _state["nc_t1"] = _build_t1()
        _state["nc_e1"] = _build_e1(G, off, sumG)
        _state["nc_e2"] = _build_e2(G, off, sumG)

    id128 = np.eye(P, dtype=_bf16)

    def gather_nodes(arr, mem):
        flat = mem.reshape(-1)
        out = arr[np.maximum(flat, 0)]
        out[flat < 0] = 0
        return out

    def trace_run(key, nc, in_maps):
        r = _run(nc, in_maps, trace=_trace)
        if _times is not None:
            _times[key] = r.exec_time_ns
            if r.instructions_and_trace is not None:
                _times["_" + key + "_insts"] = r.instructions_and_trace
        return r.results

    # ---- T1 ----
    W1 = np.concatenate([Wl1, Wr1], axis=1).astype(_bf16)      # [128, 256]
    B1t = _bcast_rows(np.concatenate([bl1, br1]).astype(_f32))
    t1_maps = []
    for k in range(NCORES):
        xg = gather_nodes(x, members[k]).astype(_bf16)         # [6272, 128]
        t1_maps.append({"xT": np.ascontiguousarray(xg.T),
                        "Wcat": W1, "Bcat": B1t})
    r1 = trace_run("t1", _state["nc_t1"], t1_maps)

    # assemble xl by node id, then expand per-edge slabs (data movement)
    xl_byid = np.zeros((N, F1), _bf16)
    for k in range(NCORES):
        flat = members[k].reshape(-1)
        ok = flat >= 0
        xl_byid[flat[ok]] = r1[k]["xl"][ok]

    ascale = 0.6 if LRELU_MODE == "abs" else 1.0
    att1_t = _bcast_rows(ascale * np.asarray(att1, _f32).reshape(-1)).astype(_bf16)
    bias1_t = _bcast_rows(bias1).astype(_f32)
    Wl2p = np.zeros((P, F2P), _f32)
    Wl2p.reshape(P, H, C2)[:, :, :DOUT] = np.asarray(Wl2, _f32).reshape(P, H, DOUT)
    Wr2p = np.zeros((P, F2P), _f32)
    Wr2p.reshape(P, H, C2)[:, :, :DOUT] = np.asarray(Wr2, _f32).reshape(P, H, DOUT)
    W2cat = np.ascontiguousarray(
        np.concatenate([Wl2p, Wr2p], axis=1)).astype(_bf16)    # [128,128]
    b2 = np.zeros(P, _f32)
    b2.reshape(2, H, C2)[0, :, :DOUT] = np.asarray(bl2, _f32).reshape(H, DOUT)
    b2.reshape(2, H, C2)[1, :, :DOUT] = np.asarray(br2, _f32).reshape(H, DOUT)
    B2col = np.ascontiguousarray(b2[:, None])

    e1_maps = []
    for k in range(NCORES):
        slab1 = xl_byid[g["idx"][k]]                           # [P, sumG, 128]
        e1_maps.append({"slab": slab1, "xr": r1[k]["xr"],
                        "mneg": g["mneg"][k],
                        "attT": att1_t, "biasT": bias1_t, "idT": id128,
                        "W2cat": W2cat, "B2col": B2col})
    re1 = trace_run("e1", _state["nc_e1"], e1_maps)

    # split xlr2T into xl2 (by node id) and xr2 rows (data movement)
    xl2_byid = np.zeros((N, F2P), _bf16)
    xr2 = [None] * NCORES
    for k in range(NCORES):
        lr = re1[k]["xlr2T"]                                   # [128, NOWN]
        flat = members[k].reshape(-1)
        ok = flat >= 0
        xl2_byid[flat[ok]] = lr[0:F2P].T[ok]
        xr2[k] = np.ascontiguousarray(lr[F2P:P].T)             # [NOWN, 64]

    att2p = np.zeros((H, C2), _f32)
    att2p[:, :DOUT] = ascale * np.asarray(att2, _f32)
    att2_t = _bcast_rows(att2p.reshape(-1)).astype(_bf16)      # [128, 64]
    bias2p = np.zeros(F2P, _f32)
    bias2p.reshape(H, C2)[:, :DOUT] = np.asarray(bias2, _f32).reshape(H, DOUT)
    bias2_t = _bcast_rows(bias2p)

    e2_maps = []
    for k in range(NCORES):
        slab2 = xl2_byid[g["idx"][k]]                          # [P, sumG, 64]
        e2_maps.append({"slab": slab2, "xr": xr2[k],
                        "mneg": g["mneg"][k],
                        "attT": att2_t, "biasT": bias2_t, "idT": id128})
    re2 = trace_run("e2", _state["nc_e2"], e2_maps)

    out = np.zeros((N, FOUT), _f32)
    for k in range(NCORES):
        flat = members[k].reshape(-1)
        ok = flat >= 0
        out[flat[ok]] = re2[k]["out"][ok]
    return out
